# revision 24
# baseline (speedup 1.0000x reference)
"""BoundaryLoss Trainium2 kernel (8 NeuronCores, SPMD, strip-replicated).

Layout: core c owns output column block [128c, 128c+128). The host hands
each core a strip of every input row covering its block plus a margin of
w columns on each side (w = bucketed max in-row nearest-background
distance, measured exactly on the host as in the previous revision).
Row-local EDT distances never exceed w at the central columns, so each
core can run the full row pass locally — no AllToAll at all, which in the
prior revision serialized ~70us of collective latency ahead of the column
pass.

Pipeline (per core):
  1. Row pass on [128, 8*W] fp16 strips (W = 128+2w; partition p, block b
     holds image row 128b+p). One forward + one reverse
     tensor_tensor_scan per image; the scan chains across block
     boundaries, but any carried-in state reaches a central column with
     value > w and so never wins (margin absorbs it).
  2. PE-transpose the central 128 columns of each block (g, fp16), square
     on the PSUM->SBUF evacuation (ACT), assembling g2^T [128 cols, 1024
     rows] directly — all overlapped with the other image's row pass.
  3. Column min-plus D2[j,i] = min_dd (dd^2 + g2T[j, i+dd]) over
     |dd| <= w on DVE in fp16 when w <= 44 (integers <= 2048 are fp16-
     exact; candidates in (2048, 4096] round by <= 1, a <= 0.05% error),
     f32 (STT pairs) otherwise. Odd shifts read a one-element-shifted
     copy to keep 4-byte alignment for the DVE 2x mode.
  4. Per-image global max via one small AllReduce (a dummy AllReduce at
     t=0 absorbs this runtime's ~55us first-collective barrier under the
     compute), then a short fp16 tail: masks compare unnormalized
     d = sqrt(D2) against 0.1*(max+1e-6), diff/abs/masked partial sums
     with fused accumulate; host sums the 8 partial pairs.
"""
import os
import sys

import numpy as np

for _p in ("/opt/trn_rl_repo", "/root/.axon_site/_ro/trn_rl_repo"):
    if os.path.isdir(_p) and _p not in sys.path:
        sys.path.append(_p)

import concourse.bacc as bacc
import concourse.tile as tile
from concourse import mybir
from concourse.bass_utils import run_bass_kernel_spmd

F32 = mybir.dt.float32
FP16 = mybir.dt.float16
I32 = mybir.dt.int32
AF = mybir.ActivationFunctionType
ALU = mybir.AluOpType
AX = mybir.AxisListType

H = 1024          # image height/width
P = 128           # partitions / rows per block / cols per core block
NB = 8            # row blocks per strip (H / P)
NCORES = 8
BIG = 1.0e4
INF = 1.0e9       # f32 sentinel
HINF = 60000.0    # fp16 sentinel (fp16 max normal is 65504)
FP16_WMAX = 44    # fp16 col pass iff w <= 44 (g^2, dd^2 <= 1936 exact)

_BUCKETS = (8, 10, 12, 14, 16, 18, 20, 22, 24, 26, 28, 32, 36, 40, 44,
            48, 56, 64, 80, 96, 128, 160, 192, 256, 320)


def _col_pass(tc, m, w, gTp, gB, persist, work):
    """Windowed min-plus; returns acc tile [P, H] (fp16 or f32).

    acc[j, i] = min_{|dd| <= w} (dd^2 + gTp[j, w + i + dd]); gTp is
    INF-padded by w on both sides. Entirely on DVE (tensor ops are
    rejected on Pool in this compiler build).
    """
    nc = tc.nc
    use16 = gB is not None

    if use16:
        def shifted(off):  # AP of width H at element offset `off` of gTp
            if off % 2 == 0:
                return gTp[:, off:off + H]
            return gB[:, off - 1:off - 1 + H]
    else:
        def shifted(off):
            return gTp[:, off:off + H]

    acc = persist.tile([P, H], FP16 if use16 else F32, tag=f"acc{m}")
    # Plain TT gets the DVE 2x mode for 16-bit and single-src TS gets 4x,
    # while the fused STT has no fast uop — so for fp16 a 3-op pairwise
    # form beats 2 STTs per dd. dd=1 folds the d=0 term.
    if use16:
        for dd in range(1, w + 1):
            tmp = work.tile([P, H], FP16, tag=f"pm{m}_{dd % 3}")
            nc.vector.tensor_tensor(tmp[:], shifted(w + dd), shifted(w - dd),
                                    ALU.min)
            nc.vector.tensor_scalar_add(tmp[:], tmp[:], float(dd * dd))
            nc.vector.tensor_tensor(
                acc[:], shifted(w) if dd == 1 else acc[:], tmp[:], ALU.min)
    else:
        for dd in range(1, w + 1):
            c = float(dd * dd)
            nc.vector.scalar_tensor_tensor(
                acc[:], shifted(w + dd), c,
                shifted(w) if dd == 1 else acc[:], ALU.add, ALU.min)
            nc.vector.scalar_tensor_tensor(
                acc[:], shifted(w - dd), c, acc[:], ALU.add, ALU.min)
    return acc


def _body(tc, w_gt, w_pred, gts, prs, partials):
    nc = tc.nc
    rg = [list(range(NCORES))]
    ws = (w_gt, w_pred)
    srcs = (gts, prs)
    use16s = tuple(w <= FP16_WMAX for w in ws)

    with tc.tile_pool(name="const", bufs=1) as const, \
         tc.tile_pool(name="work", bufs=2) as work, \
         tc.tile_pool(name="persist", bufs=1) as persist, \
         tc.tile_pool(name="ps", bufs=1, space="PSUM") as ps, \
         tc.tile_pool(name="dram", bufs=1, space="DRAM") as dram:

        # ---- input DMA (both strips in flight immediately) ----
        # The DMA trigger instructions cost ~0.7us each on the issuing
        # queue, so the two strips go out on the two HWDGE queues (SP and
        # ACT) in parallel, ahead of everything else.
        strips = []
        for m, eng in ((0, nc.sync), (1, nc.scalar)):
            wd = NB * (P + 2 * ws[m])
            s = persist.tile([P, wd], FP16, tag=f"strip{m}")
            for q in range(4):
                eng.dma_start(s[q * 32:(q + 1) * 32, :],
                              srcs[m][q * 32:(q + 1) * 32, :])
            strips.append(s)

        # ---- collective plumbing ----
        # Observed CC-stream behavior on this runtime: an automatic
        # barrier runs ~21.3us -> ~56-70us whether or not a collective has
        # been triggered yet; the first collective starts at
        # max(barrier_end, trigger) + ~11.5us and runs ~10us when its
        # trigger predates the barrier end but ~21us when triggered after
        # it; a collective_compute instruction blocks its issuing engine
        # queue until the collective completes. So: per-image AllReduces,
        # the first triggered right after the (shorter) gt column pass
        # ~48us — early enough for the fast path — doing useful work
        # instead of a dummy warm-up; the second queues behind it.
        ar_ins = [dram.tile([1, 1], F32, name=f"ari{m}", tag=f"ari{m}")
                  for m in range(2)]
        ar_outs = [nc.dram_tensor(f"ar_out_sh{m}", [1, NCORES], F32,
                                  addr_space="Shared") for m in range(2)]

        # ---- constants (DVE is idle while the strips stream in) ----
        io = const.tile([P, P], I32)
        nc.gpsimd.iota(io[:], [[1, P]], base=0, channel_multiplier=-1)
        ident = const.tile([P, P], F32)
        nc.vector.tensor_scalar(ident[:], io[:], 0, None, ALU.is_equal)
        identh = const.tile([P, P], FP16)
        nc.scalar.copy(identh[:], ident[:])
        ones1 = const.tile([1, P], F32)
        nc.vector.memset(ones1[:], 1.0)
        onesc = const.tile([P, 1], F32)
        nc.vector.memset(onesc[:], 1.0)
        maxwd = max(NB * (P + 2 * w) for w in ws)
        onesh = const.tile([P, maxwd], FP16)
        nc.vector.memset(onesh[:], 1.0)

        # ================= phase 1: row pass =================
        gs = []
        for m in range(2):
            w = ws[m]
            wd = NB * (P + 2 * w)
            s = strips[m]
            # foreground -> HINF, background -> 0. Host pre-scales inputs
            # by 1e30 (saturating fp16) so `> 0` is the fg test for both
            # images and fp16 underflow cannot flip tiny positives.
            z = work.tile([P, wd], FP16, tag=f"z{m}")
            nc.vector.tensor_scalar(z[:], s[:], 0.0, HINF, ALU.is_gt,
                                    ALU.mult)
            dl = work.tile([P, wd], FP16, tag=f"dl{m}")
            nc.vector.tensor_tensor_scan(dl[:], onesh[:, :wd], z[:], INF,
                                         ALU.add, ALU.min)
            dr = work.tile([P, wd], FP16, tag=f"dr{m}")
            nc.vector.tensor_tensor_scan(dr[:, ::-1], onesh[:, :wd],
                                         z[:, ::-1], INF, ALU.add, ALU.min)
            g = work.tile([P, wd], FP16, tag=f"g{m}")
            nc.vector.tensor_tensor(g[:], dl[:], dr[:], ALU.min)
            gs.append(g)

        # ============ phase 2: transpose + square into g2^T ============
        gTps = []
        gBs = []
        for m in range(2):
            w = ws[m]
            use16 = use16s[m]
            dt = FP16 if use16 else F32
            inf = HINF if use16 else INF
            gw = H + 2 * w
            gTp = persist.tile([P, gw], dt, tag=f"gtp{m}")
            nc.vector.memset(gTp[:, :w], inf)
            nc.vector.memset(gTp[:, w + H:], inf)
            W = P + 2 * w
            for b in range(NB):
                pt = ps.tile([P, P], FP16, tag="pt", bufs=4)
                nc.tensor.transpose(pt[:], gs[m][:, b * W + w:b * W + w + P],
                                    identh[:])
                nc.scalar.activation(gTp[:, w + b * P:w + (b + 1) * P], pt[:],
                                     AF.Square)
            if use16:
                # odd shifts read a one-element-shifted copy so the AP
                # stays 4-byte-aligned for the DVE 2x fp16 mode
                gB = persist.tile([P, gw], FP16, tag=f"gb{m}")
                nc.scalar.copy(gB[:, :gw - 1], gTp[:, 1:])
                nc.vector.memset(gB[:, gw - 1:], inf)
            else:
                gB = None
            gTps.append(gTp)
            gBs.append(gB)

        # ====== phase 3: column min-plus + per-image max + AllReduce ======
        # DVE queue order: col0, mx0, mxr0, col1, mx1, mxr1 — the tiny
        # partition-max of image 0 rides between the passes so AllReduce#0
        # triggers at ~48us, well before the CC barrier ends.
        ys = []
        mxrs = []
        for m in range(2):
            acc = _col_pass(tc, m, ws[m], gTps[m], gBs[m], persist, work)
            mxp = work.tile([P, 1], F32, tag=f"mxp{m}")
            nc.vector.reduce_max(mxp[:], acc[:], axis=AX.X)
            pmx = ps.tile([1, P], F32, tag="pmx")
            nc.tensor.transpose(pmx[:], mxp[:], ident[:])
            mxr = work.tile([1, 1], F32, tag=f"mxr{m}")
            nc.vector.reduce_max(mxr[:], pmx[:], axis=AX.X)
            mxrs.append(mxr)
            nc.sync.dma_start(ar_ins[m][0:1, 0:1], mxr[:])
            nc.gpsimd.collective_compute(
                "AllGather", ALU.bypass, replica_groups=rg,
                ins=[ar_ins[m][:, :].opt()], outs=[ar_outs[m][:, :].opt()])
            # unnormalized distances, precomputed before AllReduce#1 ends
            y = persist.tile([P, H], FP16, tag=f"y{m}")
            nc.scalar.activation(y[:], acc[:], AF.Sqrt)
            ys.append(y)

        # ============ phase 4: per-image inv = 1/(sqrt(max)+1e-6) ============
        # Image 0's chain (and a0 below) runs while AllReduce#1 is in
        # flight; image 1's is the post-collective critical path. The DVE
        # bits are emitted after both column passes so the in-order DVE
        # queue never stalls on a collective result mid-compute.
        def inv_chain(m):
            gmx8 = work.tile([1, NCORES], F32, tag=f"gmx8{m}")
            nc.sync.dma_start(gmx8[:], ar_outs[m][0:1, :])
            gmx = work.tile([1, 1], F32, tag=f"gmx{m}")
            nc.vector.reduce_max(gmx[:], gmx8[:], axis=AX.X)
            msq = work.tile([1, 1], F32, tag=f"msq{m}")
            nc.scalar.activation(msq[:], gmx[:], AF.Sqrt)
            s1 = work.tile([1, 1], F32, tag=f"s1{m}")
            nc.vector.tensor_scalar_add(s1[:], msq[:], 1e-6)
            nc.vector.reciprocal(s1[:], s1[:])
            pb = ps.tile([P, 1], F32, tag="pb")
            nc.tensor.matmul(pb[:], ones1[:], s1[:])
            invb = work.tile([P, 1], F32, tag=f"invb{m}")
            nc.scalar.copy(invb[:], pb[:])
            return invb

        # ================= phase 5: normalize + masked mean =================
        # Image 0's inv chain and a0 (on ACT; scale is a per-partition AP)
        # overlap AllReduce#1; image 1's chain and a1 (DVE) are the
        # post-collective critical path.
        invb0 = inv_chain(0)
        a0 = work.tile([P, H], FP16, tag="a0")
        nc.scalar.activation(a0[:], ys[0][:], AF.Copy, scale=invb0[:, 0:1])
        invb1 = inv_chain(1)
        a1 = work.tile([P, H], FP16, tag="a1")
        nc.vector.tensor_scalar(a1[:], ys[1][:], invb1[:, 0:1], None,
                                ALU.mult)
        # (a0 < 0.1) | (a1 < 0.1)  ==  min(a0, a1) < 0.1; the masked-|diff|
        # sum and the mask count ride the ACT accumulators (|mk*df| =
        # mk*|df|), keeping only 5 ops on the post-collective DVE path.
        mk = work.tile([P, H], FP16, tag="mk")
        nc.vector.tensor_tensor(mk[:], a0[:], a1[:], ALU.min)
        nc.vector.tensor_scalar(mk[:], mk[:], 0.1, None, ALU.is_lt)
        df = work.tile([P, H], FP16, tag="df")
        nc.vector.tensor_tensor(df[:], a0[:], a1[:], ALU.subtract)
        nc.vector.tensor_tensor(df[:], df[:], mk[:], ALU.mult)
        s12 = work.tile([P, 2], F32, tag="s12")
        da = work.tile([P, H], FP16, tag="da")
        nc.scalar.activation(da[:], df[:], AF.Abs, accum_out=s12[:, 0:1])
        cnt = work.tile([P, H], FP16, tag="cnt")
        nc.scalar.activation(cnt[:], mk[:], AF.Copy, accum_out=s12[:, 1:2])
        # partition-dim sum via PE: [1,2] = ones[128,1]^T @ s12[128,2]
        pv = ps.tile([1, 2], F32, tag="pv")
        nc.tensor.matmul(pv[:], onesc[:], s12[:])
        pvs = work.tile([1, 2], F32, tag="pvs")
        nc.scalar.copy(pvs[:], pv[:])
        nc.sync.dma_start(partials[:, :], pvs[:])


def _build(w_gt, w_pred):
    nc = bacc.Bacc("TRN2", target_bir_lowering=False, debug=False,
                   num_devices=NCORES)
    gts = nc.dram_tensor("gts", [P, NB * (P + 2 * w_gt)], FP16,
                         kind="ExternalInput")
    prs = nc.dram_tensor("prs", [P, NB * (P + 2 * w_pred)], FP16,
                         kind="ExternalInput")
    partials = nc.dram_tensor("partials", [1, 2], F32, kind="ExternalOutput")
    with tile.TileContext(nc) as tc:
        _body(tc, w_gt, w_pred, gts, prs, partials)
    nc.compile()
    return nc


_PROGRAMS = {}


def _program(*key):
    if key not in _PROGRAMS:
        _PROGRAMS[key] = _build(*key)
    return _PROGRAMS[key]


def _row_gmax(fg):
    """Max over pixels of the in-row distance to the nearest background
    pixel (clamped to BIG). This equals the exact column-pass window bound."""
    idx = np.arange(fg.shape[1], dtype=np.float64)
    zero = ~fg
    left = np.maximum.accumulate(np.where(zero, idx, -np.inf), axis=1)
    right = np.minimum.accumulate(np.where(zero, idx, np.inf)[:, ::-1],
                                  axis=1)[:, ::-1]
    g = np.minimum(np.minimum(idx - left, right - idx), BIG)
    return float(g.max())


def _bucket(gmax):
    need = min(int(np.ceil(gmax)), H - 1)
    for b in _BUCKETS:
        if b >= need:
            return b
    raise NotImplementedError(
        f"row gmax {gmax} exceeds the supported strip margin {_BUCKETS[-1]}")


def _strips(img, w):
    """Per-core fp16 strips [128, 8*(128+2w)]: strip[c][p, b*(128+2w)+q] =
    scaled img[128*b + p, 128*c - w + q], fg-padded outside the image."""
    x = np.asarray(img, np.float32) * 1e30
    pad = np.full((H, w), np.float32(1e30))
    xp = np.concatenate([pad, x, pad], axis=1)
    W = P + 2 * w
    out = []
    for c in range(NCORES):
        b = xp[:, c * P:c * P + W].astype(np.float16)
        out.append(np.ascontiguousarray(
            b.reshape(NB, P, W).transpose(1, 0, 2).reshape(P, NB * W)))
    return out


def _run(pred, gt, trace=False):
    pred = np.ascontiguousarray(np.asarray(pred), dtype=np.float32)
    gt = np.ascontiguousarray(np.asarray(gt), dtype=np.float32)
    assert pred.shape == (H, H) and gt.shape == (H, H)
    w_gt = _bucket(_row_gmax(gt != 0))
    w_pred = _bucket(_row_gmax(pred > 0))
    nc = _program(w_gt, w_pred)
    sg = _strips(gt, w_gt)
    sp = _strips(pred, w_pred)
    in_maps = [{"gts": sg[c], "prs": sp[c]} for c in range(NCORES)]
    res = run_bass_kernel_spmd(nc, in_maps, list(range(NCORES)), trace=trace)
    tot = np.zeros(2, np.float64)
    for r in res.results:
        tot += np.asarray(r["partials"], np.float64).reshape(-1)[:2]
    loss = np.float32(tot[0] / max(tot[1], 1.0))
    return loss, res


def kernel(pred, gt):
    loss, _ = _run(pred, gt)
    return loss


# revision 25
# speedup vs baseline: 1.1359x; 1.1359x over previous
"""BoundaryLoss Trainium2 kernel (8 NeuronCores, SPMD, strip-replicated).

Layout: core c owns output column block [128c, 128c+128). The host hands
each core a strip of every input row covering its block plus a margin of
w columns on each side (w = bucketed max in-row nearest-background
distance, measured exactly on the host as in the previous revision).
Row-local EDT distances never exceed w at the central columns, so each
core can run the full row pass locally — no AllToAll at all, which in the
prior revision serialized ~70us of collective latency ahead of the column
pass.

Pipeline (per core):
  1. Row pass on [128, 8*W] fp16 strips (W = 128+2w; partition p, block b
     holds image row 128b+p). One forward + one reverse
     tensor_tensor_scan per image; the scan chains across block
     boundaries, but any carried-in state reaches a central column with
     value > w and so never wins (margin absorbs it).
  2. PE-transpose the central 128 columns of each block (g, fp16), square
     on the PSUM->SBUF evacuation (ACT), assembling g2^T [128 cols, 1024
     rows] directly — all overlapped with the other image's row pass.
  3. Column min-plus D2[j,i] = min_dd (dd^2 + g2T[j, i+dd]) over
     |dd| <= w on DVE in fp16 when w <= 44 (integers <= 2048 are fp16-
     exact; candidates in (2048, 4096] round by <= 1, a <= 0.05% error),
     f32 (STT pairs) otherwise. Odd shifts read a one-element-shifted
     copy to keep 4-byte alignment for the DVE 2x mode.
  4. Per-image global max via one small AllReduce (a dummy AllReduce at
     t=0 absorbs this runtime's ~55us first-collective barrier under the
     compute), then a short fp16 tail: masks compare unnormalized
     d = sqrt(D2) against 0.1*(max+1e-6), diff/abs/masked partial sums
     with fused accumulate; host sums the 8 partial pairs.
"""
import os
import sys

import numpy as np

for _p in ("/opt/trn_rl_repo", "/root/.axon_site/_ro/trn_rl_repo"):
    if os.path.isdir(_p) and _p not in sys.path:
        sys.path.append(_p)

import concourse.bacc as bacc
import concourse.tile as tile
from concourse import mybir
from concourse.bass_utils import run_bass_kernel_spmd

F32 = mybir.dt.float32
FP16 = mybir.dt.float16
I32 = mybir.dt.int32
AF = mybir.ActivationFunctionType
ALU = mybir.AluOpType
AX = mybir.AxisListType

H = 1024          # image height/width
P = 128           # partitions / rows per block / cols per core block
NB = 8            # row blocks per strip (H / P)
NCORES = 8
BIG = 1.0e4
INF = 1.0e9       # f32 sentinel
HINF = 60000.0    # fp16 sentinel (fp16 max normal is 65504)
FP16_WMAX = 44    # fp16 col pass iff w <= 44 (g^2, dd^2 <= 1936 exact)

_BUCKETS = (8, 10, 12, 14, 16, 18, 20, 22, 24, 26, 28, 32, 36, 40, 44,
            48, 56, 64, 80, 96, 128, 160, 192, 256, 320)


def _col_pass(tc, m, w, gTp, gB, persist, work):
    """Windowed min-plus; returns acc tile [P, H] (fp16 or f32).

    acc[j, i] = min_{|dd| <= w} (dd^2 + gTp[j, w + i + dd]); gTp is
    INF-padded by w on both sides. Entirely on DVE (tensor ops are
    rejected on Pool in this compiler build).
    """
    nc = tc.nc
    use16 = gB is not None

    if use16:
        def shifted(off):  # AP of width H at element offset `off` of gTp
            if off % 2 == 0:
                return gTp[:, off:off + H]
            return gB[:, off - 1:off - 1 + H]
    else:
        def shifted(off):
            return gTp[:, off:off + H]

    acc = persist.tile([P, H], FP16 if use16 else F32, tag=f"acc{m}")
    # Plain TT gets the DVE 2x mode for 16-bit and single-src TS gets 4x,
    # while the fused STT has no fast uop — so for fp16 a 3-op pairwise
    # form beats 2 STTs per dd. dd=1 folds the d=0 term.
    if use16:
        for dd in range(1, w + 1):
            tmp = work.tile([P, H], FP16, tag=f"pm{m}_{dd % 3}")
            nc.vector.tensor_tensor(tmp[:], shifted(w + dd), shifted(w - dd),
                                    ALU.min)
            nc.vector.tensor_scalar_add(tmp[:], tmp[:], float(dd * dd))
            nc.vector.tensor_tensor(
                acc[:], shifted(w) if dd == 1 else acc[:], tmp[:], ALU.min)
    else:
        for dd in range(1, w + 1):
            c = float(dd * dd)
            nc.vector.scalar_tensor_tensor(
                acc[:], shifted(w + dd), c,
                shifted(w) if dd == 1 else acc[:], ALU.add, ALU.min)
            nc.vector.scalar_tensor_tensor(
                acc[:], shifted(w - dd), c, acc[:], ALU.add, ALU.min)
    return acc


def _body(tc, w_gt, w_pred, gts, prs, partials):
    nc = tc.nc
    rg = [list(range(NCORES))]
    ws = (w_gt, w_pred)
    srcs = (gts, prs)
    use16s = tuple(w <= FP16_WMAX for w in ws)

    with tc.tile_pool(name="const", bufs=1) as const, \
         tc.tile_pool(name="work", bufs=2) as work, \
         tc.tile_pool(name="persist", bufs=1) as persist, \
         tc.tile_pool(name="tail", bufs=1) as tail, \
         tc.tile_pool(name="ps", bufs=1, space="PSUM") as ps, \
         tc.tile_pool(name="dram", bufs=1, space="DRAM") as dram:

        # ---- input DMA (both strips in flight immediately) ----
        # The DMA trigger instructions cost ~0.7us each on the issuing
        # queue, so the two strips go out on the two HWDGE queues (SP and
        # ACT) in parallel, ahead of everything else.
        strips = []
        for m, eng in ((0, nc.sync), (1, nc.scalar)):
            wd = NB * (P + 2 * ws[m])
            s = persist.tile([P, wd], FP16, tag=f"strip{m}")
            for q in range(4):
                eng.dma_start(s[q * 32:(q + 1) * 32, :],
                              srcs[m][q * 32:(q + 1) * 32, :])
            strips.append(s)

        # ---- collective plumbing ----
        # Observed CC-stream behavior on this runtime: an automatic
        # barrier runs ~21.3us -> ~56-70us whether or not a collective has
        # been triggered yet; the first collective starts at
        # max(barrier_end, trigger) + ~11.5us and runs ~10us when its
        # trigger predates the barrier end but ~21us when triggered after
        # it; a collective_compute instruction blocks its issuing engine
        # queue until the collective completes. So: per-image AllReduces,
        # the first triggered right after the (shorter) gt column pass
        # ~48us — early enough for the fast path — doing useful work
        # instead of a dummy warm-up; the second queues behind it.
        ar_ins = [dram.tile([1, 1], F32, name=f"ari{m}", tag=f"ari{m}")
                  for m in range(2)]
        ar_outs = [nc.dram_tensor(f"ar_out_sh{m}", [1, NCORES], F32,
                                  addr_space="Shared") for m in range(2)]

        # ---- constants (DVE is idle while the strips stream in) ----
        io = const.tile([P, P], I32)
        nc.gpsimd.iota(io[:], [[1, P]], base=0, channel_multiplier=-1)
        ident = const.tile([P, P], F32)
        nc.vector.tensor_scalar(ident[:], io[:], 0, None, ALU.is_equal)
        identh = const.tile([P, P], FP16)
        nc.scalar.copy(identh[:], ident[:])
        ones1 = const.tile([1, P], F32)
        nc.vector.memset(ones1[:], 1.0)
        onesc = const.tile([P, 1], F32)
        nc.vector.memset(onesc[:], 1.0)
        maxwd = max(NB * (P + 2 * w) for w in ws)
        onesh = const.tile([P, maxwd], FP16)
        nc.vector.memset(onesh[:], 1.0)

        # ================= phase 1: row pass =================
        gs = []
        for m in range(2):
            w = ws[m]
            wd = NB * (P + 2 * w)
            s = strips[m]
            # foreground -> HINF, background -> 0. Host pre-scales inputs
            # by 1e30 (saturating fp16) so `> 0` is the fg test for both
            # images and fp16 underflow cannot flip tiny positives.
            z = work.tile([P, wd], FP16, tag=f"z{m}")
            nc.vector.tensor_scalar(z[:], s[:], 0.0, HINF, ALU.is_gt,
                                    ALU.mult)
            dl = work.tile([P, wd], FP16, tag=f"dl{m}")
            nc.vector.tensor_tensor_scan(dl[:], onesh[:, :wd], z[:], INF,
                                         ALU.add, ALU.min)
            dr = work.tile([P, wd], FP16, tag=f"dr{m}")
            nc.vector.tensor_tensor_scan(dr[:, ::-1], onesh[:, :wd],
                                         z[:, ::-1], INF, ALU.add, ALU.min)
            g = work.tile([P, wd], FP16, tag=f"g{m}")
            nc.vector.tensor_tensor(g[:], dl[:], dr[:], ALU.min)
            gs.append(g)

        # ============ phase 2: transpose + square into g2^T ============
        gTps = []
        gBs = []
        for m in range(2):
            w = ws[m]
            use16 = use16s[m]
            dt = FP16 if use16 else F32
            inf = HINF if use16 else INF
            gw = H + 2 * w
            gTp = persist.tile([P, gw], dt, tag=f"gtp{m}")
            nc.vector.memset(gTp[:, :w], inf)
            nc.vector.memset(gTp[:, w + H:], inf)
            W = P + 2 * w
            for b in range(NB):
                pt = ps.tile([P, P], FP16, tag="pt", bufs=4)
                nc.tensor.transpose(pt[:], gs[m][:, b * W + w:b * W + w + P],
                                    identh[:])
                nc.scalar.activation(gTp[:, w + b * P:w + (b + 1) * P], pt[:],
                                     AF.Square)
            if use16:
                # odd shifts read a one-element-shifted copy so the AP
                # stays 4-byte-aligned for the DVE 2x fp16 mode
                gB = persist.tile([P, gw], FP16, tag=f"gb{m}")
                nc.scalar.copy(gB[:, :gw - 1], gTp[:, 1:])
                nc.vector.memset(gB[:, gw - 1:], inf)
            else:
                gB = None
            gTps.append(gTp)
            gBs.append(gB)

        # ====== phase 3: column min-plus + per-image max + AllReduce ======
        # DVE queue order: col0, mx0, mxr0, col1, mx1, mxr1 — the tiny
        # partition-max of image 0 rides between the passes so AllReduce#0
        # triggers at ~48us, well before the CC barrier ends.
        ys = []
        mxrs = []
        for m in range(2):
            acc = _col_pass(tc, m, ws[m], gTps[m], gBs[m], persist, work)
            mxp = tail.tile([P, 1], F32, tag=f"mxp{m}")
            nc.vector.reduce_max(mxp[:], acc[:], axis=AX.X)
            pmx = ps.tile([1, P], F32, tag="pmx")
            nc.tensor.transpose(pmx[:], mxp[:], ident[:])
            mxr = tail.tile([1, 1], F32, tag=f"mxr{m}")
            nc.vector.reduce_max(mxr[:], pmx[:], axis=AX.X)
            mxrs.append(mxr)
            nc.sync.dma_start(ar_ins[m][0:1, 0:1], mxr[:])
            nc.gpsimd.collective_compute(
                "AllGather", ALU.bypass, replica_groups=rg,
                ins=[ar_ins[m][:, :].opt()], outs=[ar_outs[m][:, :].opt()])
            # unnormalized distances, precomputed before AllReduce#1 ends
            y = persist.tile([P, H], FP16, tag=f"y{m}")
            nc.scalar.activation(y[:], acc[:], AF.Sqrt)
            ys.append(y)

        # ============ phase 4: per-image inv = 1/(sqrt(max)+1e-6) ============
        # Image 0's chain (and a0 below) runs while AllReduce#1 is in
        # flight; image 1's is the post-collective critical path. The DVE
        # bits are emitted after both column passes so the in-order DVE
        # queue never stalls on a collective result mid-compute.
        def inv_chain(m):
            gmx8 = tail.tile([1, NCORES], F32, tag=f"gmx8{m}")
            nc.sync.dma_start(gmx8[:], ar_outs[m][0:1, :])
            gmx = tail.tile([1, 1], F32, tag=f"gmx{m}")
            nc.vector.reduce_max(gmx[:], gmx8[:], axis=AX.X)
            msq = tail.tile([1, 1], F32, tag=f"msq{m}")
            nc.scalar.activation(msq[:], gmx[:], AF.Sqrt)
            s1 = tail.tile([1, 1], F32, tag=f"s1{m}")
            nc.vector.tensor_scalar_add(s1[:], msq[:], 1e-6)
            nc.vector.reciprocal(s1[:], s1[:])
            pb = ps.tile([P, 1], F32, tag="pb")
            nc.tensor.matmul(pb[:], ones1[:], s1[:])
            invb = tail.tile([P, 1], F32, tag=f"invb{m}")
            nc.scalar.copy(invb[:], pb[:])
            return invb

        # ================= phase 5: normalize + masked mean =================
        # Image 0's inv chain and a0 (on ACT; scale is a per-partition AP)
        # overlap AllReduce#1; image 1's chain and a1 (DVE) are the
        # post-collective critical path.
        invb0 = inv_chain(0)
        a0 = tail.tile([P, H], FP16, tag="a0")
        nc.scalar.activation(a0[:], ys[0][:], AF.Copy, scale=invb0[:, 0:1])
        invb1 = inv_chain(1)
        a1 = tail.tile([P, H], FP16, tag="a1")
        nc.vector.tensor_scalar(a1[:], ys[1][:], invb1[:, 0:1], None,
                                ALU.mult)
        # (a0 < 0.1) | (a1 < 0.1)  ==  min(a0, a1) < 0.1; the masked-|diff|
        # sum and the mask count ride the ACT accumulators (|mk*df| =
        # mk*|df|), keeping only 5 ops on the post-collective DVE path.
        mk = tail.tile([P, H], FP16, tag="mk")
        nc.vector.tensor_tensor(mk[:], a0[:], a1[:], ALU.min)
        nc.vector.tensor_scalar(mk[:], mk[:], 0.1, None, ALU.is_lt)
        df = tail.tile([P, H], FP16, tag="df")
        nc.vector.tensor_tensor(df[:], a0[:], a1[:], ALU.subtract)
        nc.vector.tensor_tensor(df[:], df[:], mk[:], ALU.mult)
        s12 = tail.tile([P, 2], F32, tag="s12")
        da = tail.tile([P, H], FP16, tag="da")
        nc.scalar.activation(da[:], df[:], AF.Abs, accum_out=s12[:, 0:1])
        cnt = tail.tile([P, H], FP16, tag="cnt")
        nc.scalar.activation(cnt[:], mk[:], AF.Copy, accum_out=s12[:, 1:2])
        # partition-dim sum via PE: [1,2] = ones[128,1]^T @ s12[128,2]
        pv = ps.tile([1, 2], F32, tag="pv")
        nc.tensor.matmul(pv[:], onesc[:], s12[:])
        pvs = tail.tile([1, 2], F32, tag="pvs")
        nc.scalar.copy(pvs[:], pv[:])
        nc.sync.dma_start(partials[:, :], pvs[:])


def _build(w_gt, w_pred):
    nc = bacc.Bacc("TRN2", target_bir_lowering=False, debug=False,
                   num_devices=NCORES)
    gts = nc.dram_tensor("gts", [P, NB * (P + 2 * w_gt)], FP16,
                         kind="ExternalInput")
    prs = nc.dram_tensor("prs", [P, NB * (P + 2 * w_pred)], FP16,
                         kind="ExternalInput")
    partials = nc.dram_tensor("partials", [1, 2], F32, kind="ExternalOutput")
    with tile.TileContext(nc) as tc:
        _body(tc, w_gt, w_pred, gts, prs, partials)
    nc.compile()
    return nc


_PROGRAMS = {}


def _program(*key):
    if key not in _PROGRAMS:
        _PROGRAMS[key] = _build(*key)
    return _PROGRAMS[key]


def _row_gmax(fg):
    """Max over pixels of the in-row distance to the nearest background
    pixel (clamped to BIG). This equals the exact column-pass window bound."""
    idx = np.arange(fg.shape[1], dtype=np.float64)
    zero = ~fg
    left = np.maximum.accumulate(np.where(zero, idx, -np.inf), axis=1)
    right = np.minimum.accumulate(np.where(zero, idx, np.inf)[:, ::-1],
                                  axis=1)[:, ::-1]
    g = np.minimum(np.minimum(idx - left, right - idx), BIG)
    return float(g.max())


def _bucket(gmax):
    need = min(int(np.ceil(gmax)), H - 1)
    for b in _BUCKETS:
        if b >= need:
            return b
    raise NotImplementedError(
        f"row gmax {gmax} exceeds the supported strip margin {_BUCKETS[-1]}")


def _strips(img, w):
    """Per-core fp16 strips [128, 8*(128+2w)]: strip[c][p, b*(128+2w)+q] =
    scaled img[128*b + p, 128*c - w + q], fg-padded outside the image."""
    x = np.asarray(img, np.float32) * 1e30
    pad = np.full((H, w), np.float32(1e30))
    xp = np.concatenate([pad, x, pad], axis=1)
    W = P + 2 * w
    out = []
    for c in range(NCORES):
        b = xp[:, c * P:c * P + W].astype(np.float16)
        out.append(np.ascontiguousarray(
            b.reshape(NB, P, W).transpose(1, 0, 2).reshape(P, NB * W)))
    return out


def _run(pred, gt, trace=False):
    pred = np.ascontiguousarray(np.asarray(pred), dtype=np.float32)
    gt = np.ascontiguousarray(np.asarray(gt), dtype=np.float32)
    assert pred.shape == (H, H) and gt.shape == (H, H)
    w_gt = _bucket(_row_gmax(gt != 0))
    w_pred = _bucket(_row_gmax(pred > 0))
    nc = _program(w_gt, w_pred)
    sg = _strips(gt, w_gt)
    sp = _strips(pred, w_pred)
    in_maps = [{"gts": sg[c], "prs": sp[c]} for c in range(NCORES)]
    res = run_bass_kernel_spmd(nc, in_maps, list(range(NCORES)), trace=trace)
    tot = np.zeros(2, np.float64)
    for r in res.results:
        tot += np.asarray(r["partials"], np.float64).reshape(-1)[:2]
    loss = np.float32(tot[0] / max(tot[1], 1.0))
    return loss, res


def kernel(pred, gt):
    loss, _ = _run(pred, gt)
    return loss
